# revision 16
# baseline (speedup 1.0000x reference)
"""Trainium2 Bass kernel for a top-2 MoE layer (T=2048, H=2048, I=1408, E=8).

Strategy: expert-parallel over 8 NeuronCores. The host dispatches tokens:
for each expert e it gathers the tokens routed to e (~480 of 2048, padded to
a capacity of 512), so each core runs a dense [C,H]x[2I,H]->silu*mul->[C,H]
FFN for its expert — a 4x FLOP saving over dense all-experts compute. The
host then combines per-expert outputs with the routing weights (the
"all-to-all dispatch/combine" of the sharding hint, done host-side since
full inputs/outputs live on the host).

Device kernel (per core), all in a transposed layout so no on-device
transposes are needed:
  stage 1: guT[2816, C] = w13 @ xT         (22 x 16 matmuls, K-tiles of 128)
  stage 2: actT[1408, C] = silu(gT) * uT   (ScalarE Silu + VectorE mul)
  stage 3: yT[2048, C] = w2 @ actT         (16 x 11 matmuls)
Matmuls run in float32r (full fp32 data, ~bf16 PE speed at N>=256).
Weights are pre-tiled on the host into the exact SBUF layout so every DMA
is a large contiguous transfer.
"""

import sys

if "/opt/trn_rl_repo" not in sys.path:
    sys.path.insert(0, "/opt/trn_rl_repo")

import os
import numpy as np
from contextlib import ExitStack

import concourse.bass as bass
import concourse.tile as tile
from concourse import bacc, mybir

T, H, I, E, K = 2048, 2048, 1408, 8, 2
C = 512                      # token capacity per expert per pass
HT = H // 128                # 16 K-tiles over H
IT = I // 128                # 11 K-tiles over I
BT = 2 * I // 128            # 22 row-blocks of guT

import ml_dtypes

MODE = os.environ.get("KERNEL_DTYPE", "f16")
if MODE == "bf16":
    DT = mybir.dt.bfloat16
    NP_DT = ml_dtypes.bfloat16
elif MODE == "f16":
    # fp16: 2-byte DMA + FWL like bf16, but 11 mantissa bits (~8x better
    # quantization error); all values here are < 100 so no range risk
    DT = mybir.dt.float16
    NP_DT = np.float16
else:
    DT = mybir.dt.float32r   # matmul dtype (fp32 bits, fast PE mode)
    NP_DT = np.float32       # host-side array dtype matching DT

_cache: dict = {}


def _build_nc():
    """Build + compile the per-core FFN program (same program on all cores)."""
    nc = bacc.Bacc("TRN2", target_bir_lowering=False, debug=False, num_devices=E)
    x_d = nc.dram_tensor("x_sb", [HT, 128, C], DT, kind="ExternalInput")
    w13_d = nc.dram_tensor("w13_sb", [BT, 128, HT * 128], DT, kind="ExternalInput")
    w2_d = nc.dram_tensor("w2_sb", [HT, 128, IT * 128], DT, kind="ExternalInput")
    y_d = nc.dram_tensor("y_sb", [HT, 128, C], mybir.dt.float32, kind="ExternalOutput")

    AF = mybir.ActivationFunctionType
    F32 = mybir.dt.float32

    with tile.TileContext(nc) as tc, ExitStack() as ctx:
        xp = ctx.enter_context(tc.tile_pool(name="x", bufs=1))
        wp = ctx.enter_context(tc.tile_pool(name="w", bufs=6))
        ap = ctx.enter_context(tc.tile_pool(name="act", bufs=1))
        sp = ctx.enter_context(tc.tile_pool(name="tmp", bufs=2))
        psg = ctx.enter_context(
            tc.tile_pool(name="psg", bufs=4, space=bass.MemorySpace.PSUM)
        )
        psy = ctx.enter_context(
            tc.tile_pool(name="psy", bufs=3, space=bass.MemorySpace.PSUM)
        )

        # first g/u weight blocks issued before the x tiles so the first
        # matmul chain isn't queued behind 4MB of token DMAs
        wgu0 = []
        for i, b in enumerate((0, IT)):
            w0 = wp.tile([128, HT * 128], DT, tag="w13", name=f"w0_{b}")
            (nc.sync if i == 0 else nc.scalar).dma_start(w0[:], w13_d.ap()[b])
            wgu0.append(w0)
        x_t = []
        for k in range(HT):
            xt = xp.tile([128, C], DT, tag=f"x{k}", name=f"x{k}")
            nc.gpsimd.dma_start(xt[:], x_d.ap()[k])
            x_t.append(xt)

        # stage 1+2: guT blocks (g row-block m pairs with u row-block m+IT)
        act_t = []
        for m in range(IT):
            if m == 0:
                wg, wu = wgu0
                g_w = [wg[:, k * 128 : (k + 1) * 128] for k in range(HT)]
                u_w = [wu[:, k * 128 : (k + 1) * 128] for k in range(HT)]
            else:
                wg = wp.tile([128, HT * 128], DT, tag="w13")
                nc.sync.dma_start(wg[:], w13_d.ap()[m])
                wu = wp.tile([128, HT * 128], DT, tag="w13")
                nc.scalar.dma_start(wu[:], w13_d.ap()[m + IT])
                g_w = [wg[:, k * 128 : (k + 1) * 128] for k in range(HT)]
                u_w = [wu[:, k * 128 : (k + 1) * 128] for k in range(HT)]
            g_ps = psg.tile([128, C], F32, tag="ps")
            u_ps = psg.tile([128, C], F32, tag="ps")
            for k in range(HT):
                nc.tensor.matmul(
                    g_ps[:], g_w[k], x_t[k][:],
                    start=(k == 0), stop=(k == HT - 1),
                )
            for k in range(HT):
                nc.tensor.matmul(
                    u_ps[:], u_w[k], x_t[k][:],
                    start=(k == 0), stop=(k == HT - 1),
                )
            sg = sp.tile([128, C], F32, tag="sg")
            nc.scalar.activation(sg[:], g_ps[:], AF.Silu)
            at = ap.tile([128, C], DT, tag=f"act{m}")
            nc.vector.tensor_mul(at[:], sg[:], u_ps[:])
            act_t.append(at)

        # stage 3: yT row-blocks
        for m in range(HT):
            w2t = wp.tile([128, IT * 128], DT, tag="w2")
            (nc.sync if m % 2 == 0 else nc.scalar).dma_start(w2t[:], w2_d.ap()[m])
            y_ps = psy.tile([128, C], F32, tag="y")
            for k in range(IT):
                nc.tensor.matmul(
                    y_ps[:], w2t[:, k * 128 : (k + 1) * 128],
                    act_t[k][:],
                    start=(k == 0), stop=(k == IT - 1),
                )
            y_sb = sp.tile([128, C], F32, tag="yout")
            nc.scalar.copy(y_sb[:], y_ps[:])
            (nc.scalar if m % 2 == 0 else nc.sync).dma_start(y_d.ap()[m], y_sb[:])

    nc.compile()
    return nc


def _get_nc():
    if "nc" not in _cache:
        _cache["nc"] = _build_nc()
    return _cache["nc"]


def _prep_weights(w13, w2):
    """Pre-tile weights into the SBUF layout the kernel DMAs verbatim.

    w13_sb[e, b, p, k*128+c] = w13[e, b*128+c, k*128+p]   (b: guT row-block)
    w2_sb [e, m, p, k*128+c] = w2 [e, m*128+c, k*128+p]   (m: yT row-block)
    """
    w13_sb = (
        w13.reshape(E, BT, 128, HT, 128)
        .transpose(0, 1, 4, 3, 2)
        .astype(NP_DT)
        .reshape(E, BT, 128, HT * 128)
    )
    w2_sb = (
        w2.reshape(E, HT, 128, IT, 128)
        .transpose(0, 1, 4, 3, 2)
        .astype(NP_DT)
        .reshape(E, HT, 128, IT * 128)
    )
    return w13_sb, w2_sb


def kernel(
    hidden_states,
    topk_weights,
    topk_ids,
    w13,
    w2,
    num_global_tokens=None,
    max_num_tokens_per_gpu=None,
):
    from concourse.bass_utils import run_bass_kernel_spmd

    hs = np.asarray(hidden_states, dtype=np.float32)
    tw = np.asarray(topk_weights, dtype=np.float32)
    ti = np.asarray(topk_ids)
    w13 = np.asarray(w13, dtype=np.float32)
    w2 = np.asarray(w2, dtype=np.float32)

    assert hs.shape == (T, H), hs.shape
    assert w13.shape == (E, 2 * I, H), w13.shape
    assert w2.shape == (E, H, I), w2.shape

    # per-(token, expert) combine weights: sum of topk weights routed to e
    comb = np.zeros((T, E), dtype=np.float32)
    for k in range(ti.shape[1]):
        np.add.at(comb, (np.arange(T), ti[:, k]), tw[:, k])

    idxs = [np.nonzero(comb[:, e])[0] for e in range(E)]
    nchunks = max(1, -(-max(len(ix) for ix in idxs) // C))

    w13_sb, w2_sb = _prep_weights(w13, w2)
    nc = _get_nc()

    trace = bool(os.environ.get("KERNEL_PROFILE"))
    out = np.zeros((T, H), dtype=np.float32)
    for chunk in range(nchunks):
        in_maps = []
        sels = []
        for e in range(E):
            sel = idxs[e][chunk * C : (chunk + 1) * C]
            xe = np.zeros((C, H), dtype=np.float32)
            xe[: len(sel)] = hs[sel]
            x_sb = np.ascontiguousarray(xe.T.reshape(HT, 128, C)).astype(
                NP_DT, copy=False
            )
            in_maps.append({"x_sb": x_sb, "w13_sb": w13_sb[e], "w2_sb": w2_sb[e]})
            sels.append(sel)
        res = run_bass_kernel_spmd(nc, in_maps, list(range(E)), trace=trace)
        if trace and res.exec_time_ns is not None:
            print(f"HW exec time: {res.exec_time_ns} ns")
        for e in range(E):
            sel = sels[e]
            if len(sel) == 0:
                continue
            y_sb = np.asarray(res.results[e]["y_sb"], dtype=np.float32)
            ye = y_sb.reshape(H, C).T  # [C, H]
            out[sel] += comb[sel, e][:, None] * ye[: len(sel)]
    return out


# revision 17
# speedup vs baseline: 1.0712x; 1.0712x over previous
"""Trainium2 Bass kernel for a top-2 MoE layer (T=2048, H=2048, I=1408, E=8).

Strategy: expert-parallel over 8 NeuronCores. The host dispatches tokens:
for each expert e it gathers the tokens routed to e (~480 of 2048, padded to
a capacity of 512), so each core runs a dense [C,H]x[2I,H]->silu*mul->[C,H]
FFN for its expert — a 4x FLOP saving over dense all-experts compute. The
host then combines per-expert outputs with the routing weights (the
"all-to-all dispatch/combine" of the sharding hint, done host-side since
full inputs/outputs live on the host).

Device kernel (per core), all in a transposed layout so no on-device
transposes are needed:
  stage 1: guT[2816, C] = w13 @ xT         (22 x 16 matmuls, K-tiles of 128)
  stage 2: actT[1408, C] = silu(gT) * uT   (ScalarE Silu + VectorE mul)
  stage 3: yT[2048, C] = w2 @ actT         (16 x 11 matmuls)
Matmuls run in float32r (full fp32 data, ~bf16 PE speed at N>=256).
Weights are pre-tiled on the host into the exact SBUF layout so every DMA
is a large contiguous transfer.
"""

import sys

if "/opt/trn_rl_repo" not in sys.path:
    sys.path.insert(0, "/opt/trn_rl_repo")

import os
import numpy as np
from contextlib import ExitStack

import concourse.bass as bass
import concourse.tile as tile
from concourse import bacc, mybir

T, H, I, E, K = 2048, 2048, 1408, 8, 2
C = 512                      # token capacity per expert per pass
HT = H // 128                # 16 K-tiles over H
IT = I // 128                # 11 K-tiles over I
BT = 2 * I // 128            # 22 row-blocks of guT

import ml_dtypes

MODE = os.environ.get("KERNEL_DTYPE", "f16")
if MODE == "bf16":
    DT = mybir.dt.bfloat16
    NP_DT = ml_dtypes.bfloat16
elif MODE == "f16":
    # fp16: 2-byte DMA + FWL like bf16, but 11 mantissa bits (~8x better
    # quantization error); all values here are < 100 so no range risk
    DT = mybir.dt.float16
    NP_DT = np.float16
else:
    DT = mybir.dt.float32r   # matmul dtype (fp32 bits, fast PE mode)
    NP_DT = np.float32       # host-side array dtype matching DT

_cache: dict = {}


def _build_nc():
    """Build + compile the per-core FFN program (same program on all cores)."""
    nc = bacc.Bacc("TRN2", target_bir_lowering=False, debug=False, num_devices=E)
    x_d = nc.dram_tensor("x_sb", [HT, 128, C], DT, kind="ExternalInput")
    w13_d = nc.dram_tensor("w13_sb", [BT, 128, HT * 128], DT, kind="ExternalInput")
    w2_d = nc.dram_tensor("w2_sb", [HT, 128, IT * 128], DT, kind="ExternalInput")
    y_d = nc.dram_tensor("y_sb", [HT, 128, C], mybir.dt.float32, kind="ExternalOutput")

    AF = mybir.ActivationFunctionType
    F32 = mybir.dt.float32

    with tile.TileContext(nc) as tc, ExitStack() as ctx:
        xp = ctx.enter_context(tc.tile_pool(name="x", bufs=1))
        wp = ctx.enter_context(tc.tile_pool(name="w", bufs=6))
        ap = ctx.enter_context(tc.tile_pool(name="act", bufs=1))
        sp = ctx.enter_context(tc.tile_pool(name="tmp", bufs=2))
        psg = ctx.enter_context(
            tc.tile_pool(name="psg", bufs=4, space=bass.MemorySpace.PSUM)
        )
        psy = ctx.enter_context(
            tc.tile_pool(name="psy", bufs=3, space=bass.MemorySpace.PSUM)
        )

        # first g/u weight blocks issued before the x tiles so the first
        # matmul chain isn't queued behind 4MB of token DMAs
        wgu0 = []
        for i, b in enumerate((0, IT)):
            w0 = wp.tile([128, HT * 128], DT, tag="w13", name=f"w0_{b}")
            (nc.sync if i == 0 else nc.scalar).dma_start(w0[:], w13_d.ap()[b])
            wgu0.append(w0)
        x_t = []
        for k in range(HT):
            xt = xp.tile([128, C], DT, tag=f"x{k}", name=f"x{k}")
            nc.sync.dma_start(xt[:], x_d.ap()[k])
            x_t.append(xt)

        # stage 1+2: guT blocks (g row-block m pairs with u row-block m+IT)
        act_t = []
        for m in range(IT):
            if m == 0:
                wg, wu = wgu0
                g_w = [wg[:, k * 128 : (k + 1) * 128] for k in range(HT)]
                u_w = [wu[:, k * 128 : (k + 1) * 128] for k in range(HT)]
            else:
                wg = wp.tile([128, HT * 128], DT, tag="w13")
                nc.sync.dma_start(wg[:], w13_d.ap()[m])
                wu = wp.tile([128, HT * 128], DT, tag="w13")
                nc.scalar.dma_start(wu[:], w13_d.ap()[m + IT])
                g_w = [wg[:, k * 128 : (k + 1) * 128] for k in range(HT)]
                u_w = [wu[:, k * 128 : (k + 1) * 128] for k in range(HT)]
            g_ps = psg.tile([128, C], F32, tag="ps")
            u_ps = psg.tile([128, C], F32, tag="ps")
            for k in range(HT):
                nc.tensor.matmul(
                    g_ps[:], g_w[k], x_t[k][:],
                    start=(k == 0), stop=(k == HT - 1),
                )
            for k in range(HT):
                nc.tensor.matmul(
                    u_ps[:], u_w[k], x_t[k][:],
                    start=(k == 0), stop=(k == HT - 1),
                )
            sg = sp.tile([128, C], F32, tag="sg")
            nc.scalar.activation(sg[:], g_ps[:], AF.Silu)
            at = ap.tile([128, C], DT, tag=f"act{m}")
            nc.vector.tensor_mul(at[:], sg[:], u_ps[:])
            act_t.append(at)

        # stage 3: yT row-blocks
        for m in range(HT):
            w2t = wp.tile([128, IT * 128], DT, tag="w2")
            nc.sync.dma_start(w2t[:], w2_d.ap()[m])
            y_ps = psy.tile([128, C], F32, tag="y")
            for k in range(IT):
                nc.tensor.matmul(
                    y_ps[:], w2t[:, k * 128 : (k + 1) * 128],
                    act_t[k][:],
                    start=(k == 0), stop=(k == IT - 1),
                )
            y_sb = sp.tile([128, C], F32, tag="yout")
            nc.scalar.copy(y_sb[:], y_ps[:])
            nc.sync.dma_start(y_d.ap()[m], y_sb[:])

    nc.compile()
    return nc


def _get_nc():
    if "nc" not in _cache:
        _cache["nc"] = _build_nc()
    return _cache["nc"]


def _prep_weights(w13, w2):
    """Pre-tile weights into the SBUF layout the kernel DMAs verbatim.

    w13_sb[e, b, p, k*128+c] = w13[e, b*128+c, k*128+p]   (b: guT row-block)
    w2_sb [e, m, p, k*128+c] = w2 [e, m*128+c, k*128+p]   (m: yT row-block)
    """
    w13_sb = (
        w13.reshape(E, BT, 128, HT, 128)
        .transpose(0, 1, 4, 3, 2)
        .astype(NP_DT)
        .reshape(E, BT, 128, HT * 128)
    )
    w2_sb = (
        w2.reshape(E, HT, 128, IT, 128)
        .transpose(0, 1, 4, 3, 2)
        .astype(NP_DT)
        .reshape(E, HT, 128, IT * 128)
    )
    return w13_sb, w2_sb


def kernel(
    hidden_states,
    topk_weights,
    topk_ids,
    w13,
    w2,
    num_global_tokens=None,
    max_num_tokens_per_gpu=None,
):
    from concourse.bass_utils import run_bass_kernel_spmd

    hs = np.asarray(hidden_states, dtype=np.float32)
    tw = np.asarray(topk_weights, dtype=np.float32)
    ti = np.asarray(topk_ids)
    w13 = np.asarray(w13, dtype=np.float32)
    w2 = np.asarray(w2, dtype=np.float32)

    assert hs.shape == (T, H), hs.shape
    assert w13.shape == (E, 2 * I, H), w13.shape
    assert w2.shape == (E, H, I), w2.shape

    # per-(token, expert) combine weights: sum of topk weights routed to e
    comb = np.zeros((T, E), dtype=np.float32)
    for k in range(ti.shape[1]):
        np.add.at(comb, (np.arange(T), ti[:, k]), tw[:, k])

    idxs = [np.nonzero(comb[:, e])[0] for e in range(E)]
    nchunks = max(1, -(-max(len(ix) for ix in idxs) // C))

    w13_sb, w2_sb = _prep_weights(w13, w2)
    nc = _get_nc()

    trace = bool(os.environ.get("KERNEL_PROFILE"))
    out = np.zeros((T, H), dtype=np.float32)
    for chunk in range(nchunks):
        in_maps = []
        sels = []
        for e in range(E):
            sel = idxs[e][chunk * C : (chunk + 1) * C]
            xe = np.zeros((C, H), dtype=np.float32)
            xe[: len(sel)] = hs[sel]
            x_sb = np.ascontiguousarray(xe.T.reshape(HT, 128, C)).astype(
                NP_DT, copy=False
            )
            in_maps.append({"x_sb": x_sb, "w13_sb": w13_sb[e], "w2_sb": w2_sb[e]})
            sels.append(sel)
        res = run_bass_kernel_spmd(nc, in_maps, list(range(E)), trace=trace)
        if trace and res.exec_time_ns is not None:
            print(f"HW exec time: {res.exec_time_ns} ns")
        for e in range(E):
            sel = sels[e]
            if len(sel) == 0:
                continue
            y_sb = np.asarray(res.results[e]["y_sb"], dtype=np.float32)
            ye = y_sb.reshape(H, C).T  # [C, H]
            out[sel] += comb[sel, e][:, None] * ye[: len(sel)]
    return out


# revision 18
# speedup vs baseline: 1.1014x; 1.0282x over previous
"""Trainium2 Bass kernel for a top-2 MoE layer (T=2048, H=2048, I=1408, E=8).

Strategy: expert-parallel over 8 NeuronCores. The host dispatches tokens:
for each expert e it gathers the tokens routed to e (~480 of 2048, padded to
a capacity of 512), so each core runs a dense [C,H]x[2I,H]->silu*mul->[C,H]
FFN for its expert — a 4x FLOP saving over dense all-experts compute. The
host then combines per-expert outputs with the routing weights (the
"all-to-all dispatch/combine" of the sharding hint, done host-side since
full inputs/outputs live on the host).

Device kernel (per core), all in a transposed layout so no on-device
transposes are needed:
  stage 1: guT[2816, C] = w13 @ xT         (22 x 16 matmuls, K-tiles of 128)
  stage 2: actT[1408, C] = silu(gT) * uT   (ScalarE Silu + VectorE mul)
  stage 3: yT[2048, C] = w2 @ actT         (16 x 11 matmuls)
Matmuls run in float32r (full fp32 data, ~bf16 PE speed at N>=256).
Weights are pre-tiled on the host into the exact SBUF layout so every DMA
is a large contiguous transfer.
"""

import sys

if "/opt/trn_rl_repo" not in sys.path:
    sys.path.insert(0, "/opt/trn_rl_repo")

import os
import numpy as np
from contextlib import ExitStack

import concourse.bass as bass
import concourse.tile as tile
from concourse import bacc, mybir

T, H, I, E, K = 2048, 2048, 1408, 8, 2
C = 512                      # token capacity per expert per pass
HT = H // 128                # 16 K-tiles over H
IT = I // 128                # 11 K-tiles over I
BT = 2 * I // 128            # 22 row-blocks of guT

import ml_dtypes

MODE = os.environ.get("KERNEL_DTYPE", "f16")
if MODE == "bf16":
    DT = mybir.dt.bfloat16
    NP_DT = ml_dtypes.bfloat16
elif MODE == "f16":
    # fp16: 2-byte DMA + FWL like bf16, but 11 mantissa bits (~8x better
    # quantization error); all values here are < 100 so no range risk
    DT = mybir.dt.float16
    NP_DT = np.float16
else:
    DT = mybir.dt.float32r   # matmul dtype (fp32 bits, fast PE mode)
    NP_DT = np.float32       # host-side array dtype matching DT

_cache: dict = {}


def _build_nc():
    """Build + compile the per-core FFN program (same program on all cores)."""
    nc = bacc.Bacc("TRN2", target_bir_lowering=False, debug=False, num_devices=E)
    x_d = nc.dram_tensor("x_sb", [HT, 128, C], DT, kind="ExternalInput")
    w13_d = nc.dram_tensor("w13_sb", [BT, 128, HT * 128], DT, kind="ExternalInput")
    w2_d = nc.dram_tensor("w2_sb", [HT, 128, IT * 128], DT, kind="ExternalInput")
    y_d = nc.dram_tensor("y_sb", [HT, 128, C], mybir.dt.float32, kind="ExternalOutput")

    AF = mybir.ActivationFunctionType
    F32 = mybir.dt.float32

    with tile.TileContext(nc) as tc, ExitStack() as ctx:
        xp = ctx.enter_context(tc.tile_pool(name="x", bufs=1))
        wp = ctx.enter_context(tc.tile_pool(name="w", bufs=6))
        ap = ctx.enter_context(tc.tile_pool(name="act", bufs=1))
        sp = ctx.enter_context(tc.tile_pool(name="tmp", bufs=2))
        psg = ctx.enter_context(
            tc.tile_pool(name="psg", bufs=4, space=bass.MemorySpace.PSUM)
        )
        psy = ctx.enter_context(
            tc.tile_pool(name="psy", bufs=3, space=bass.MemorySpace.PSUM)
        )

        # first g/u weight blocks issued before the x tiles so the first
        # matmul chain isn't queued behind 4MB of token DMAs
        wgu0 = []
        for i, b in enumerate((0, IT)):
            w0 = wp.tile([128, HT * 128], DT, tag="w13", name=f"w0_{b}")
            nc.sync.dma_start(w0[:], w13_d.ap()[b])
            wgu0.append(w0)
        x_t = []
        for k in range(HT):
            xt = xp.tile([128, C], DT, tag=f"x{k}", name=f"x{k}")
            nc.sync.dma_start(xt[:], x_d.ap()[k])
            x_t.append(xt)

        # stage 1+2: guT blocks (g row-block m pairs with u row-block m+IT)
        act_t = []
        for m in range(IT):
            if m == 0:
                wg, wu = wgu0
                g_w = [wg[:, k * 128 : (k + 1) * 128] for k in range(HT)]
                u_w = [wu[:, k * 128 : (k + 1) * 128] for k in range(HT)]
            else:
                wg = wp.tile([128, HT * 128], DT, tag="w13")
                nc.sync.dma_start(wg[:], w13_d.ap()[m])
                wu = wp.tile([128, HT * 128], DT, tag="w13")
                nc.sync.dma_start(wu[:], w13_d.ap()[m + IT])
                g_w = [wg[:, k * 128 : (k + 1) * 128] for k in range(HT)]
                u_w = [wu[:, k * 128 : (k + 1) * 128] for k in range(HT)]
            g_ps = psg.tile([128, C], F32, tag="ps")
            u_ps = psg.tile([128, C], F32, tag="ps")
            for k in range(HT):
                nc.tensor.matmul(
                    g_ps[:], g_w[k], x_t[k][:],
                    start=(k == 0), stop=(k == HT - 1),
                )
            for k in range(HT):
                nc.tensor.matmul(
                    u_ps[:], u_w[k], x_t[k][:],
                    start=(k == 0), stop=(k == HT - 1),
                )
            sg = sp.tile([128, C], F32, tag="sg")
            nc.scalar.activation(sg[:], g_ps[:], AF.Silu)
            at = ap.tile([128, C], DT, tag=f"act{m}")
            nc.vector.tensor_mul(at[:], sg[:], u_ps[:])
            act_t.append(at)

        # stage 3: yT row-blocks
        for m in range(HT):
            w2t = wp.tile([128, IT * 128], DT, tag="w2")
            nc.sync.dma_start(w2t[:], w2_d.ap()[m])
            y_ps = psy.tile([128, C], F32, tag="y")
            for k in range(IT):
                nc.tensor.matmul(
                    y_ps[:], w2t[:, k * 128 : (k + 1) * 128],
                    act_t[k][:],
                    start=(k == 0), stop=(k == IT - 1),
                )
            y_sb = sp.tile([128, C], F32, tag="yout")
            nc.scalar.copy(y_sb[:], y_ps[:])
            nc.sync.dma_start(y_d.ap()[m], y_sb[:])

    nc.compile()
    return nc


def _get_nc():
    if "nc" not in _cache:
        _cache["nc"] = _build_nc()
    return _cache["nc"]


def _prep_weights(w13, w2):
    """Pre-tile weights into the SBUF layout the kernel DMAs verbatim.

    w13_sb[e, b, p, k*128+c] = w13[e, b*128+c, k*128+p]   (b: guT row-block)
    w2_sb [e, m, p, k*128+c] = w2 [e, m*128+c, k*128+p]   (m: yT row-block)
    """
    w13_sb = (
        w13.reshape(E, BT, 128, HT, 128)
        .transpose(0, 1, 4, 3, 2)
        .astype(NP_DT)
        .reshape(E, BT, 128, HT * 128)
    )
    w2_sb = (
        w2.reshape(E, HT, 128, IT, 128)
        .transpose(0, 1, 4, 3, 2)
        .astype(NP_DT)
        .reshape(E, HT, 128, IT * 128)
    )
    return w13_sb, w2_sb


def kernel(
    hidden_states,
    topk_weights,
    topk_ids,
    w13,
    w2,
    num_global_tokens=None,
    max_num_tokens_per_gpu=None,
):
    from concourse.bass_utils import run_bass_kernel_spmd

    hs = np.asarray(hidden_states, dtype=np.float32)
    tw = np.asarray(topk_weights, dtype=np.float32)
    ti = np.asarray(topk_ids)
    w13 = np.asarray(w13, dtype=np.float32)
    w2 = np.asarray(w2, dtype=np.float32)

    assert hs.shape == (T, H), hs.shape
    assert w13.shape == (E, 2 * I, H), w13.shape
    assert w2.shape == (E, H, I), w2.shape

    # per-(token, expert) combine weights: sum of topk weights routed to e
    comb = np.zeros((T, E), dtype=np.float32)
    for k in range(ti.shape[1]):
        np.add.at(comb, (np.arange(T), ti[:, k]), tw[:, k])

    idxs = [np.nonzero(comb[:, e])[0] for e in range(E)]
    nchunks = max(1, -(-max(len(ix) for ix in idxs) // C))

    w13_sb, w2_sb = _prep_weights(w13, w2)
    nc = _get_nc()

    trace = bool(os.environ.get("KERNEL_PROFILE"))
    out = np.zeros((T, H), dtype=np.float32)
    for chunk in range(nchunks):
        in_maps = []
        sels = []
        for e in range(E):
            sel = idxs[e][chunk * C : (chunk + 1) * C]
            xe = np.zeros((C, H), dtype=np.float32)
            xe[: len(sel)] = hs[sel]
            x_sb = np.ascontiguousarray(xe.T.reshape(HT, 128, C)).astype(
                NP_DT, copy=False
            )
            in_maps.append({"x_sb": x_sb, "w13_sb": w13_sb[e], "w2_sb": w2_sb[e]})
            sels.append(sel)
        res = run_bass_kernel_spmd(nc, in_maps, list(range(E)), trace=trace)
        if trace and res.exec_time_ns is not None:
            print(f"HW exec time: {res.exec_time_ns} ns")
        for e in range(E):
            sel = sels[e]
            if len(sel) == 0:
                continue
            y_sb = np.asarray(res.results[e]["y_sb"], dtype=np.float32)
            ye = y_sb.reshape(H, C).T  # [C, H]
            out[sel] += comb[sel, e][:, None] * ye[: len(sel)]
    return out


# revision 19
# speedup vs baseline: 1.1058x; 1.0040x over previous
"""Trainium2 Bass kernel for a top-2 MoE layer (T=2048, H=2048, I=1408, E=8).

Strategy: expert-parallel over 8 NeuronCores. The host dispatches tokens:
for each expert e it gathers the tokens routed to e (~480 of 2048, padded to
a capacity of 512), so each core runs a dense [C,H]x[2I,H]->silu*mul->[C,H]
FFN for its expert — a 4x FLOP saving over dense all-experts compute. The
host then combines per-expert outputs with the routing weights (the
"all-to-all dispatch/combine" of the sharding hint, done host-side since
full inputs/outputs live on the host).

Device kernel (per core), all in a transposed layout so no on-device
transposes are needed:
  stage 1: guT[2816, C] = w13 @ xT         (22 x 16 matmuls, K-tiles of 128)
  stage 2: actT[1408, C] = silu(gT) * uT   (ScalarE Silu + VectorE mul)
  stage 3: yT[2048, C] = w2 @ actT         (16 x 11 matmuls)
Matmuls run in float32r (full fp32 data, ~bf16 PE speed at N>=256).
Weights are pre-tiled on the host into the exact SBUF layout so every DMA
is a large contiguous transfer.
"""

import sys

if "/opt/trn_rl_repo" not in sys.path:
    sys.path.insert(0, "/opt/trn_rl_repo")

import os
import numpy as np
from contextlib import ExitStack

import concourse.bass as bass
import concourse.tile as tile
from concourse import bacc, mybir

T, H, I, E, K = 2048, 2048, 1408, 8, 2
C = 512                      # token capacity per expert per pass
HT = H // 128                # 16 K-tiles over H
IT = I // 128                # 11 K-tiles over I
BT = 2 * I // 128            # 22 row-blocks of guT

import ml_dtypes

MODE = os.environ.get("KERNEL_DTYPE", "f16")
if MODE == "bf16":
    DT = mybir.dt.bfloat16
    NP_DT = ml_dtypes.bfloat16
elif MODE == "f16":
    # fp16: 2-byte DMA + FWL like bf16, but 11 mantissa bits (~8x better
    # quantization error); all values here are < 100 so no range risk
    DT = mybir.dt.float16
    NP_DT = np.float16
else:
    DT = mybir.dt.float32r   # matmul dtype (fp32 bits, fast PE mode)
    NP_DT = np.float32       # host-side array dtype matching DT

_cache: dict = {}


def _build_nc():
    """Build + compile the per-core FFN program (same program on all cores)."""
    nc = bacc.Bacc("TRN2", target_bir_lowering=False, debug=False, num_devices=E)
    x_d = nc.dram_tensor("x_sb", [HT, 128, C], DT, kind="ExternalInput")
    w13_d = nc.dram_tensor("w13_sb", [BT, 128, HT * 128], DT, kind="ExternalInput")
    w2_d = nc.dram_tensor("w2_sb", [HT, 128, IT * 128], DT, kind="ExternalInput")
    y_d = nc.dram_tensor("y_sb", [HT, 128, C], mybir.dt.float32, kind="ExternalOutput")

    AF = mybir.ActivationFunctionType
    F32 = mybir.dt.float32

    with tile.TileContext(nc) as tc, ExitStack() as ctx:
        xp = ctx.enter_context(tc.tile_pool(name="x", bufs=1))
        wp = ctx.enter_context(tc.tile_pool(name="w", bufs=6))
        ap = ctx.enter_context(tc.tile_pool(name="act", bufs=1))
        sp = ctx.enter_context(tc.tile_pool(name="tmp", bufs=2))
        psg = ctx.enter_context(
            tc.tile_pool(name="psg", bufs=4, space=bass.MemorySpace.PSUM)
        )
        psy = ctx.enter_context(
            tc.tile_pool(name="psy", bufs=3, space=bass.MemorySpace.PSUM)
        )

        # first g/u weight blocks issued before the x tiles so the first
        # matmul chain isn't queued behind 4MB of token DMAs
        # Issue order matters: DMAs drain roughly FIFO, so interleave the
        # first three m-iterations' weight blocks into the x burst so each
        # arrives just before its matmul chain needs it.
        wgu = {}
        def _load_w13(m):
            wg = wp.tile([128, HT * 128], DT, tag="w13", name=f"wg{m}")
            nc.sync.dma_start(wg[:], w13_d.ap()[m])
            wu = wp.tile([128, HT * 128], DT, tag="w13", name=f"wu{m}")
            nc.sync.dma_start(wu[:], w13_d.ap()[m + IT])
            wgu[m] = (wg, wu)
        _load_w13(0)
        x_t = []
        for k in range(HT):
            xt = xp.tile([128, C], DT, tag=f"x{k}", name=f"x{k}")
            nc.sync.dma_start(xt[:], x_d.ap()[k])
            x_t.append(xt)
            if k == 3:
                _load_w13(1)
            elif k == 7:
                _load_w13(2)

        # stage 1+2: guT blocks (g row-block m pairs with u row-block m+IT)
        act_t = []
        for m in range(IT):
            if m not in wgu:
                _load_w13(m)
            wg, wu = wgu.pop(m)
            g_w = [wg[:, k * 128 : (k + 1) * 128] for k in range(HT)]
            u_w = [wu[:, k * 128 : (k + 1) * 128] for k in range(HT)]
            g_ps = psg.tile([128, C], F32, tag="ps")
            u_ps = psg.tile([128, C], F32, tag="ps")
            for k in range(HT):
                nc.tensor.matmul(
                    g_ps[:], g_w[k], x_t[k][:],
                    start=(k == 0), stop=(k == HT - 1),
                )
            for k in range(HT):
                nc.tensor.matmul(
                    u_ps[:], u_w[k], x_t[k][:],
                    start=(k == 0), stop=(k == HT - 1),
                )
            sg = sp.tile([128, C], F32, tag="sg")
            nc.scalar.activation(sg[:], g_ps[:], AF.Silu)
            at = ap.tile([128, C], DT, tag=f"act{m}")
            nc.vector.tensor_mul(at[:], sg[:], u_ps[:])
            act_t.append(at)

        # stage 3: yT row-blocks
        for m in range(HT):
            w2t = wp.tile([128, IT * 128], DT, tag="w2")
            nc.sync.dma_start(w2t[:], w2_d.ap()[m])
            y_ps = psy.tile([128, C], F32, tag="y")
            for k in range(IT):
                nc.tensor.matmul(
                    y_ps[:], w2t[:, k * 128 : (k + 1) * 128],
                    act_t[k][:],
                    start=(k == 0), stop=(k == IT - 1),
                )
            y_sb = sp.tile([128, C], F32, tag="yout")
            nc.scalar.copy(y_sb[:], y_ps[:])
            nc.sync.dma_start(y_d.ap()[m], y_sb[:])

    nc.compile()
    return nc


def _get_nc():
    if "nc" not in _cache:
        _cache["nc"] = _build_nc()
    return _cache["nc"]


def _prep_weights(w13, w2):
    """Pre-tile weights into the SBUF layout the kernel DMAs verbatim.

    w13_sb[e, b, p, k*128+c] = w13[e, b*128+c, k*128+p]   (b: guT row-block)
    w2_sb [e, m, p, k*128+c] = w2 [e, m*128+c, k*128+p]   (m: yT row-block)
    """
    w13_sb = (
        w13.reshape(E, BT, 128, HT, 128)
        .transpose(0, 1, 4, 3, 2)
        .astype(NP_DT)
        .reshape(E, BT, 128, HT * 128)
    )
    w2_sb = (
        w2.reshape(E, HT, 128, IT, 128)
        .transpose(0, 1, 4, 3, 2)
        .astype(NP_DT)
        .reshape(E, HT, 128, IT * 128)
    )
    return w13_sb, w2_sb


def kernel(
    hidden_states,
    topk_weights,
    topk_ids,
    w13,
    w2,
    num_global_tokens=None,
    max_num_tokens_per_gpu=None,
):
    from concourse.bass_utils import run_bass_kernel_spmd

    hs = np.asarray(hidden_states, dtype=np.float32)
    tw = np.asarray(topk_weights, dtype=np.float32)
    ti = np.asarray(topk_ids)
    w13 = np.asarray(w13, dtype=np.float32)
    w2 = np.asarray(w2, dtype=np.float32)

    assert hs.shape == (T, H), hs.shape
    assert w13.shape == (E, 2 * I, H), w13.shape
    assert w2.shape == (E, H, I), w2.shape

    # per-(token, expert) combine weights: sum of topk weights routed to e
    comb = np.zeros((T, E), dtype=np.float32)
    for k in range(ti.shape[1]):
        np.add.at(comb, (np.arange(T), ti[:, k]), tw[:, k])

    idxs = [np.nonzero(comb[:, e])[0] for e in range(E)]
    nchunks = max(1, -(-max(len(ix) for ix in idxs) // C))

    w13_sb, w2_sb = _prep_weights(w13, w2)
    nc = _get_nc()

    trace = bool(os.environ.get("KERNEL_PROFILE"))
    out = np.zeros((T, H), dtype=np.float32)
    for chunk in range(nchunks):
        in_maps = []
        sels = []
        for e in range(E):
            sel = idxs[e][chunk * C : (chunk + 1) * C]
            xe = np.zeros((C, H), dtype=np.float32)
            xe[: len(sel)] = hs[sel]
            x_sb = np.ascontiguousarray(xe.T.reshape(HT, 128, C)).astype(
                NP_DT, copy=False
            )
            in_maps.append({"x_sb": x_sb, "w13_sb": w13_sb[e], "w2_sb": w2_sb[e]})
            sels.append(sel)
        res = run_bass_kernel_spmd(nc, in_maps, list(range(E)), trace=trace)
        if trace and res.exec_time_ns is not None:
            print(f"HW exec time: {res.exec_time_ns} ns")
        for e in range(E):
            sel = sels[e]
            if len(sel) == 0:
                continue
            y_sb = np.asarray(res.results[e]["y_sb"], dtype=np.float32)
            ye = y_sb.reshape(H, C).T  # [C, H]
            out[sel] += comb[sel, e][:, None] * ye[: len(sel)]
    return out
